# revision 22
# baseline (speedup 1.0000x reference)
"""Dynamic depthwise 3x3 conv (per-pixel weights) on 8 Trainium2 NeuronCores.

Problem:
  x:            [4, 64, 256, 256]  f32
  conv_weights: [4, 576, 256, 256] f32  (= [4, 64ch * 9tap, 256, 256])
  out[n,c,h,w] = sum_k w[n, c*9+k, h, w] * xpad[n, c, h+ki, w+kj],  k=(ki,kj)

Sharding: pure data parallel over (batch n, H-half) -> 8 shards.

This version moves all streams to bf16 (halves HBM traffic; rel err ~0.5%
vs the 2e-2 gate) and replaces the 1-elem/cycle segmented MAC with a custom
2x-mode DVE op (SEG_MAC2_ANT) that retires 2 bf16 MACs/cycle:

  * outputs are processed in adjacent pairs (j, j+1); each DVE cycle reads a
    packed bf16 pair of weights (one tap for both outputs) and a packed pair
    of x values
  * x is stored in an "overlapped pairs" layout x3[2e]=x[e], x3[2e+1]=x[e+1],
    which turns the overlapping 3-tap windows into contiguous aligned reads
  * two independent accumulators (even/odd j) live on different ALU blocks;
    subdim pages of 6 elements = 3 cycles per 3-tap dot; writes are gated to
    the page-final cycle (out_last_subdim_enable), one packed write per pair
  * the instruction carries perf_max=1 in byte-36[7:6] (the firmware decodes
    it; bass.py never sets it, so a scoped monkeypatch injects it)

Per tile (J=2048 outputs/partition): 3 per-dh segmacs (3 cycles/pair each)
+ 2 bf16 tensor_adds combine the dh partials. Weight-edge taps (wd=0,dw=0 /
wd=255,dw=2) are zeroed host-side so width wrap-around contributes nothing.
"""

import sys

sys.path.insert(0, "/opt/trn_rl_repo")

import numpy as np
import ml_dtypes

import concourse.bass as bass
import concourse.bacc as bacc
import concourse.tile as tile
from concourse import mybir
from concourse.bass_utils import run_bass_kernel_spmd

import concourse.dve_ops as dve_ops
import concourse.bass_isa as bass_isa
from concourse.dve_spec import Spec, Src0, Src1
from concourse.dve_uop import (
    ENABLE,
    AluInp,
    AluOp,
    DelayInp,
    DveOpSpec,
    InpSel,
    OutPath,
    OutSel,
    Trigger,
    UopConfig,
    UopDpConfig,
)

# ---------------------------------------------------------------------------
# SEG_MAC2_ANT: j-paired segmented 3-tap MAC, 2 bf16 MACs/cycle.
#   out[p, 2*jp + q] = sum_dw w[p, 6*jp + 2*dw + q] * x3[p, 4*jp + 2*dw + q]
# ---------------------------------------------------------------------------

OP_NAME = "SEG_MAC2_ANT"

_SRC = Trigger.SRC_TENSOR_DONE
_SUB = Trigger.SUB_DIM_DONE
_CNT = Trigger.COUNT
_NON = Trigger.NONE


def _dp(mode):
    """mode: 'seed' (acc<-0), 'steady' (acc+=p), 'step' (acc=0+p).
    Chains: c0=SRC_0 then p_e, c1=SRC_1 then p_o, c2=SRC_0_HI then acc_e,
    c3=SRC_1_HI, c4=ZERO. acc_e at block b2, acc_o at block b7."""
    dp = [UopDpConfig() for _ in range(8)]
    for st in range(8):
        dp[st].pass_through_delay(0, 1, 2, 3, 4)
    dp[0].enable_alu(AluOp.MULTIPLY, AluInp.PREV_DELAY_0, AluInp.PREV_DELAY_1)
    dp[1].enable_alu(AluOp.MULTIPLY, AluInp.PREV_DELAY_2, AluInp.PREV_DELAY_3)
    dp[1].enable_delay_from_src(DelayInp.PREV_ALU_OUT, 0)  # capture p_e
    if mode == "seed":
        dp[2].enable_alu(AluOp.BYPASS, AluInp.PREV_DELAY_4, AluInp.PREV_DELAY_4)
    elif mode == "steady":
        dp[2].enable_alu(AluOp.ADD, AluInp.CURR_ALU_OUT, AluInp.PREV_DELAY_0)
    else:
        dp[2].enable_alu(AluOp.ADD, AluInp.PREV_DELAY_4, AluInp.PREV_DELAY_0)
    dp[2].enable_delay_from_src(DelayInp.PREV_ALU_OUT, 1)  # capture p_o
    dp[3].enable_delay_from_src(DelayInp.PREV_ALU_OUT, 2)  # capture acc_e
    for st in range(3, 7):
        dp[st].pass_through_alu()
    if mode == "seed":
        dp[7].enable_alu(AluOp.BYPASS, AluInp.PREV_DELAY_4, AluInp.PREV_DELAY_4)
    elif mode == "steady":
        dp[7].enable_alu(AluOp.ADD, AluInp.CURR_ALU_OUT, AluInp.PREV_DELAY_1)
    else:
        dp[7].enable_alu(AluOp.ADD, AluInp.PREV_DELAY_4, AluInp.PREV_DELAY_1)
    return dp


def _uop(mode, trig, nxt, repeat, consume, write):
    u = UopConfig(datapath_config=_dp(mode))
    u.enable_input(InpSel.SRC_0, 1)
    u.enable_input(InpSel.SRC_1, 2)
    u.enable_input(InpSel.SRC_0_HI, 3)
    u.enable_input(InpSel.SRC_1_HI, 4)
    u.enable_input(InpSel.ZERO, 5)
    if write:
        u.enable_output(OutSel.DELAY_2, OutPath.WR0_LO)   # acc_e
        u.enable_output(OutSel.ALU_OUT, OutPath.WR0_HI)   # acc_o (b7)
        u.out_last_subdim_enable = ENABLE
    if consume:
        u.require_inp0 = ENABLE
        u.require_inp1 = ENABLE
    u.repeat_count = repeat
    u.trigger = trig
    u.next_uop = nxt
    return u


def _program():
    return [
        _uop("seed", (_CNT, _NON, _NON), (1, 0, 0), 1, False, False),
        _uop("steady", (_SRC, _SUB, _NON), (0, 2, 0), 0, True, True),
        _uop("step", (_SRC, _SUB, _CNT), (0, 2, 1), 1, True, True),
    ]


def _segmac2_ref(in0, in1, c0, c1, c2):
    P = in0.shape[0]
    a = np.asarray(in0, np.float32).reshape(P, -1, 3, 2)
    b = np.asarray(in1, np.float32).reshape(P, -1, 3, 2)
    return (a * b).sum(axis=2)


def get_segmac2_op():
    existing = getattr(dve_ops, "_ANT_SEG_MAC2", None)
    if existing is not None:
        return existing

    spec = Spec(body=Src0 * Src1, reference=_segmac2_ref)
    op = dve_ops.DveOp(OP_NAME, spec, subdim=True, uops_sha={})
    dve_ops.OPS.append(op)
    row = dve_ops._CUSTOM_DVE_ROW_BASE + len(dve_ops.OPS) - 1
    assert row < 0x20
    dve_ops._SUB_OPCODE_FOR_NAME[OP_NAME] = row
    dve_ops.CUSTOM_DVE_SPECS[OP_NAME] = spec

    compiled = DveOpSpec(
        name=OP_NAME,
        opcode=row,
        uops=_program(),
        uops_2x=_program(),
        rd1_en=True,
        perf_max=1,
    )
    compiled.validate("v3")
    dve_ops._COMPILE_CACHE[(OP_NAME, "v3")] = compiled

    import concourse.bass as bass_mod

    orig = bass_isa.InstCustomDveAnt

    def patched(*args, **kwargs):
        if kwargs.get("op_name") == OP_NAME:
            kwargs.setdefault("perf_max", 1)
        return orig(*args, **kwargs)

    bass_isa.InstCustomDveAnt = patched
    bass_mod.bass_isa.InstCustomDveAnt = patched

    dve_ops._ANT_SEG_MAC2 = op
    return op


def window_ap(sl, dims):
    """AP over `sl`'s tensor/offset with explicit free dims [[step, count],...]."""
    import bass_rust

    return bass_rust.AP(
        sl.tensor,
        sl.offset,
        [list(sl.ap[0])] + [list(d) for d in dims],
        sl.const_val,
        sl.runtime_checks,
        sl.dep_tracking_offset,
    )


# ---------------------------------------------------------------------------
# Kernel
# ---------------------------------------------------------------------------

N, C, H, W = 4, 64, 256, 256
KW = 3
NCORES = 8
HH = H // 2          # rows per core
RB = HH // 2         # rows per partition block (64)
Rh = 8               # rows per h-tile
T = RB // Rh         # h-tiles per core (8)
J = Rh * W           # outputs per partition per tile (2048)
JP = J // 2          # output pairs (1024)
WSEG = 6 * JP        # w elems per dh chunk
WF = 3 * WSEG        # w elems per tile (9*J)
NXT = 4              # resident x tiles per core
XB = RB // NXT       # output rows per x tile (16)
XR = XB + 2          # rows per x tile incl halo (18)
X3F = 2 * XR * W + 4  # doubled x elems per x tile (+guards)
BF = mybir.dt.bfloat16
BF16 = ml_dtypes.bfloat16

_CACHE = {}


TBIG = T - 1         # full-size tiles (7)
NTL = 2              # tail micro-tiles
RhL = Rh // NTL      # rows per micro-tile (4)
JL = RhL * W         # outputs per partition per micro-tile (512)
JPL = JL // 2
WSEGL = 6 * JPL


def _build():
    op = get_segmac2_op()
    nc = bacc.Bacc("TRN2", target_bir_lowering=False, debug=False, num_devices=NCORES)
    x_in = nc.dram_tensor("x", [NXT, 128, X3F], BF, kind="ExternalInput")
    w_in = nc.dram_tensor("w", [TBIG, KW, 128, WSEG], BF, kind="ExternalInput")
    wl_in = nc.dram_tensor("wl", [NTL, KW, 128, WSEGL], BF, kind="ExternalInput")
    y_out = nc.dram_tensor("y", [TBIG, 128, J], BF, kind="ExternalOutput")
    yl_out = nc.dram_tensor("yl", [NTL, 128, JL], BF, kind="ExternalOutput")

    with tile.TileContext(nc) as tc:
        with (
            tc.tile_pool(name="xp", bufs=1) as xpool,
            tc.tile_pool(name="wp", bufs=2) as wpool,
            tc.tile_pool(name="o0", bufs=2) as o0pool,
            tc.tile_pool(name="pa", bufs=1) as papool,
            tc.tile_pool(name="pb", bufs=1) as pbpool,
        ):
            # x tiles are prefetched one step ahead of use so the initial
            # x flood doesn't starve the w stream on the scalar ring.
            xtiles = []
            for s in range(NXT):
                xt = xpool.tile([128, X3F], BF, tag=f"x{s}")
                xtiles.append(xt)
            nc.scalar.dma_start(out=xtiles[0][:], in_=x_in[0])

            kctr = [0]

            def do_tile(w_src, y_dst, r0, j, wseg):
                jp = j // 2
                s = r0 // XB
                rb = r0 - s * XB
                xt = xtiles[s]
                wt = wpool.tile([128, KW * wseg], BF, name="wt")
                for dh in range(KW):
                    eng = nc.sync if kctr[0] % 2 == 0 else nc.scalar
                    kctr[0] += 1
                    eng.dma_start(
                        out=wt[:, dh * wseg:(dh + 1) * wseg],
                        in_=w_src[dh],
                    )
                ot = o0pool.tile([128, j], BF, name="ot")
                pa = papool.tile([128, j], BF, name="pa")
                pb = pbpool.tile([128, j], BF, name="pb")
                for dh, tgt in ((0, ot), (1, pa), (2, pb)):
                    xbase = 2 * (rb + dh) * W
                    nc.vector._custom_dve(
                        op,
                        out=window_ap(tgt[:, 0:j], [[2, jp], [1, 2]]),
                        in0=window_ap(
                            wt[:, dh * wseg:(dh + 1) * wseg], [[6, jp], [1, 6]]
                        ),
                        in1=window_ap(
                            xt[:, xbase:xbase + 4 * jp + 2], [[4, jp], [1, 6]]
                        ),
                    )
                nc.vector.tensor_add(ot[:], ot[:], pa[:])
                nc.vector.tensor_add(ot[:], ot[:], pb[:])
                nc.gpsimd.dma_start(out=y_dst, in_=ot[:])

            TPX = T // NXT  # full tiles per x tile
            for t in range(TBIG):
                if t % TPX == 0 and t // TPX + 1 < NXT:
                    s_next = t // TPX + 1
                    nc.scalar.dma_start(out=xtiles[s_next][:], in_=x_in[s_next])
                do_tile(w_in[t], y_out[t], t * Rh, J, WSEG)
            for i in range(NTL):
                do_tile(wl_in[i], yl_out[i], TBIG * Rh + i * RhL, JL, WSEGL)
    nc.compile()
    return nc


def _get_nc():
    if "nc" not in _CACHE:
        _CACHE["nc"] = _build()
    return _CACHE["nc"]


def _pack_core(xh_n: np.ndarray, w5_n: np.ndarray, hf: int):
    """Repack one core's shard (bf16 inputs).

    xh_n: [C, H+2, W] H-padded x for batch n (bf16); w5_n: [C, 9, H, W] bf16.
    Returns x_blocks [NXT, 128, X3F], w_blocks [T, 128, WF].
    """
    xc = xh_n[:, hf * HH:hf * HH + HH + 2, :]          # [C, HH+2, W]
    wc = w5_n[:, :, hf * HH:(hf + 1) * HH, :]          # [C, 9, HH, W]

    # x3: doubled layout per x tile, per partition block
    xb = np.zeros((NXT, 2, C, X3F), dtype=BF16)
    L = XR * W
    for s in range(NXT):
        for hb in range(2):
            r0 = hb * RB + s * XB
            flat = xc[:, r0:r0 + XR, :].reshape(C, L)
            ext = np.zeros((C, L + 2), dtype=BF16)
            ext[:, :L] = flat
            if r0 + XR < HH + 2:
                ext[:, L:L + 2] = xc[:, r0 + XR, :2]
            xb[s, hb, :, 2:2 + 2 * L:2] = flat
            xb[s, hb, :, 3:3 + 2 * L:2] = ext[:, 1:L + 1]

    # w: [C, (dh,dw), (hb,t,r), (wp,q)] -> [t, dh, hb, c, r, wp, dw, q]
    # (dh-major so each per-dh DMA chunk is fully contiguous in HBM)
    wb = (
        wc.reshape(C, KW, KW, 2, T, Rh, W // 2, 2)
        .transpose(4, 1, 3, 0, 5, 6, 2, 7)
        .copy()
    )  # [T, dh, hb, C, r, wp, dw, q]
    # width-edge taps multiply zero padding in the reference -> zero them
    wb[:, :, :, :, :, 0, 0, 0] = 0
    wb[:, :, :, :, :, W // 2 - 1, KW - 1, 1] = 0
    w_big = np.ascontiguousarray(wb[:TBIG].reshape(TBIG, KW, 128, WSEG))
    # last full tile -> NTL micro-tiles of RhL rows
    wtail = wb[TBIG].reshape(KW, 2, C, NTL, RhL, W // 2, KW, 2)
    w_tail = np.ascontiguousarray(
        wtail.transpose(3, 0, 1, 2, 4, 5, 6, 7).reshape(NTL, KW, 128, WSEGL)
    )
    return xb.reshape(NXT, 128, X3F), w_big, w_tail


def _make_in_maps(x: np.ndarray, conv_weights: np.ndarray):
    x = np.asarray(x, dtype=np.float32).astype(BF16)
    w5 = (
        np.asarray(conv_weights, dtype=np.float32)
        .astype(BF16)
        .reshape(N, C, KW * KW, H, W)
    )
    xh = np.zeros((N, C, H + 2, W), dtype=BF16)
    xh[:, :, 1:-1, :] = x

    in_maps = []
    for i in range(NCORES):
        n, hf = divmod(i, 2)
        xb, wb, wl = _pack_core(xh[n], w5[n], hf)
        in_maps.append({"x": xb, "w": wb, "wl": wl})
    return in_maps


def kernel(x: np.ndarray, conv_weights: np.ndarray) -> np.ndarray:
    nc = _get_nc()
    in_maps = _make_in_maps(x, conv_weights)
    res = run_bass_kernel_spmd(nc, in_maps, list(range(NCORES)))
    out = np.empty((N, C, H, W), dtype=np.float32)
    for i in range(NCORES):
        n, hf = divmod(i, 2)
        yb = np.asarray(res.results[i]["y"]).reshape(TBIG, 2, C, Rh, W)
        yl = np.asarray(res.results[i]["yl"]).reshape(NTL, 2, C, RhL, W)
        # rows per block: big tiles cover rows [0, TBIG*Rh), tail the rest
        full = np.concatenate(
            [
                yb.transpose(1, 2, 0, 3, 4).reshape(2, C, TBIG * Rh, W),
                yl.transpose(1, 2, 0, 3, 4).reshape(2, C, NTL * RhL, W),
            ],
            axis=2,
        )  # [hb, C, RB, W]
        oc = full.transpose(1, 0, 2, 3).reshape(C, HH, W).astype(np.float32)
        out[n, :, hf * HH:(hf + 1) * HH, :] = oc
    return out
